# revision 16
# baseline (speedup 1.0000x reference)
"""CPC Smartpool encoder on 8 TRN2 NeuronCores (Bass/Tile, SPMD).

Sharding: core c = (sample b = c//2, time-half h = c%2). h=1 cores process the
time-REVERSED input slice with tap-reversed conv weights (mirror trick), so a
single SPMD program serves all cores; per-core differences live entirely in
the input data (x slice, weights, warp masks, h-flag).

Pipeline per core (activations [channel, time] layout, weights bf16):
  conv0: per-tile contiguous overlapping-row x loads; norm via Gram trick
    (ssq = (G@X)*X summed, G = W0 W0^T host-side), rstd via Rsqrt activation
  conv1..conv3: tap matmuls (bf16 stationary), norm: Square->ones-matmul ssq,
    Rsqrt, bcast matmul, relu*scale
  MLP: z1 = gelu(mw1^T f); z2^T = gelu(sum_k z1_k^T mw2_k) (stationary = z1
    chunks, moving = streamed mw2) -> logits via vector mul+reduce -> sigmoid
    -> imp as COLUMNS [128, 2] directly
  pair AllReduce(sum of own imp) -> cs via universal triangular masks +
    flag*(total-own) offset, * TN/total scale -> ramp -> wmat
  pooled partial = wmat^T @ f -> pair ReduceScatter(add) -> cnorm4+relu
Host reassembles [4, 512, 256].
"""

import math
import os

import numpy as np

import concourse.bass as bass
import concourse.mybir as mybir
import concourse.tile as tile
from concourse import bacc
from concourse.bass_utils import run_bass_kernel_spmd
from concourse.masks import make_identity

# ---------------------------------------------------------------- constants
B, L, C, DMLP = 4, 40960, 512, 2048
T, TN = 512, 256
EPS = 1e-5
TEMP = 1e-5

XP_LEN = 20555
T0, HP0_LEN = 4110, 4112
T1, HP1_LEN = 1027, 1028
T2, HP2_LEN = 513, 514
T3 = 256

F32 = mybir.dt.float32
FR = mybir.dt.float32r
BF = mybir.dt.bfloat16

GROUPS = [[0, 1], [2, 3], [4, 5], [6, 7]]
ACT = mybir.ActivationFunctionType


def _ttiles(total):
    """Even-width t-tiles (fp32r needs an even moving dim); the final tile is
    widened to an even size >= 4 by overlapping the previous tile."""
    tiles = []
    t0 = 0
    while total - t0 > 512:
        tiles.append((t0, 512))
        t0 += 512
    rem = total - t0
    if rem % 2 == 1 or rem < 4:
        w = max(4, rem + (rem % 2))
        tiles.append((total - w, w))
    else:
        tiles.append((t0, rem))
    return tiles


# ---------------------------------------------------------------- host prep
def _center(w):
    return w - w.mean(axis=0, keepdims=True)


def _prep_x_slices(x):
    out = []
    for b in range(B):
        xp = np.pad(np.asarray(x[b, 0], np.float32), (3, 3), mode="reflect")
        out.append([xp[0:XP_LEN].copy(), xp[20410:40965][::-1].copy()])
    return out


def _prep_conv_weights(conv_ws, bf16):
    outs = []
    for h in range(2):
        ws = []
        g0 = None
        for li, w in enumerate(conv_ws):
            wc = _center(np.asarray(w, np.float32))
            if h == 1:
                wc = wc[:, :, ::-1]
            K = wc.shape[2]
            if li == 0:
                w0 = np.ascontiguousarray(wc[:, 0, :].T)  # [10, 512]
                g0 = np.ascontiguousarray(w0 @ w0.T)  # [10, 10]
                ws.append(w0)
            else:
                arr = np.transpose(wc, (2, 1, 0)).reshape(K, 4, 128, C)
                ws.append(np.ascontiguousarray(arr.astype(bf16)))
        outs.append((ws, g0))
    return outs


def _prep_masks(h):
    j = np.arange(256)[:, None]
    r = np.arange(256)[None, :]
    if h == 0:
        mA = (j <= r).astype(np.float32)
        mB = (j < r).astype(np.float32)
    else:
        mA = (j >= r).astype(np.float32)
        mB = (j > r).astype(np.float32)
    return np.ascontiguousarray(np.stack([mA, mB]).reshape(2, 2, 128, TN))


def _prep_iota():
    return np.ascontiguousarray(
        np.broadcast_to(np.arange(TN + 1, dtype=np.float32), (128, TN + 1))
    )


# ------------------------------------------------------------ numpy fallback
def _np_reference(inputs):
    """Exact numpy port of the reference; used only when inputs do not match
    the expected identity-affine/zero-bias pattern."""
    erf = np.vectorize(math.erf, otypes=[np.float64])

    def conv(x, w, b, stride, pad):
        xp = np.pad(x, ((0, 0), (pad, pad)), mode="reflect")
        K = w.shape[2]
        Tout = (xp.shape[1] - K) // stride + 1
        out = np.zeros((w.shape[0], Tout), np.float32)
        for k in range(K):
            out += w[:, :, k] @ xp[:, k : k + stride * Tout : stride]
        return out + b[:, None]

    def cnorm(x, g, bb):
        m = x.mean(0, keepdims=True)
        v = x.var(0, ddof=1, keepdims=True)
        return (x - m) / np.sqrt(v + EPS) * g[:, None] + bb[:, None]

    def gg(z):
        return (0.5 * z * (1.0 + erf(z / np.sqrt(2.0)))).astype(np.float32)

    outs = []
    for b in range(B):
        hcur = np.asarray(inputs["x"][b], np.float32)
        for li, (s, p) in enumerate([(5, 3), (4, 2), (2, 1), (2, 1)]):
            hcur = conv(
                hcur,
                np.asarray(inputs[f"conv{li}_w"], np.float32),
                np.asarray(inputs[f"conv{li}_b"], np.float32),
                s,
                p,
            )
            hcur = np.maximum(
                cnorm(
                    hcur,
                    np.asarray(inputs[f"n{li}_w"], np.float32),
                    np.asarray(inputs[f"n{li}_b"], np.float32),
                ),
                0,
            )
        f = hcur.T
        z = gg(f @ np.asarray(inputs["mlp_w1"], np.float32) + np.asarray(inputs["mlp_b1"], np.float32))
        z = gg(z @ np.asarray(inputs["mlp_w2"], np.float32) + np.asarray(inputs["mlp_b2"], np.float32))
        logit = (z @ np.asarray(inputs["mlp_w3"], np.float32) + np.asarray(inputs["mlp_b3"], np.float32))[:, 0]
        imp = 1.0 / (1.0 + np.exp(-logit)) + TEMP
        imp = imp / imp.sum() * (T / 2)
        cs = np.cumsum(imp).astype(np.float32)
        p_ = np.maximum(cs[:, None] - np.arange(TN, dtype=np.float32)[None, :], 0.0)
        pc = np.pad(p_, ((0, 0), (0, 1)))
        d = pc[:, :-1] - pc[:, 1:]
        wm = d - np.pad(d, ((1, 0), (0, 0)))[:-1, :]
        pooled = wm.T @ f
        out = np.maximum(
            cnorm(
                pooled.T,
                np.asarray(inputs["n4_w"], np.float32),
                np.asarray(inputs["n4_b"], np.float32),
            ),
            0,
        )
        outs.append(out)
    return np.stack(outs).astype(np.float32)


def _fast_path_ok(inputs):
    try:
        if tuple(np.asarray(inputs["x"]).shape) != (B, 1, L):
            return False
        for i in range(4):
            if np.any(np.asarray(inputs[f"conv{i}_b"]) != 0):
                return False
        for i in range(3):
            if np.any(np.asarray(inputs[f"mlp_b{i + 1}"]) != 0):
                return False
        for i in range(5):
            if np.any(np.asarray(inputs[f"n{i}_w"]) != 1):
                return False
            if np.any(np.asarray(inputs[f"n{i}_b"]) != 0):
                return False
        return True
    except Exception:
        return False


# ------------------------------------------------------------ device program
_CACHE = {}


def _build_program():
    stage = int(os.environ.get("KSTAGE", "9"))
    key = ("nc", stage)
    if key in _CACHE:
        return _CACHE[key]

    nc = bacc.Bacc("TRN2", target_bir_lowering=False, debug=False, num_devices=8)

    xp_d = nc.dram_tensor("xp", [XP_LEN], FR, kind="ExternalInput")
    w0_d = nc.dram_tensor("w0", [10, C], FR, kind="ExternalInput")
    g0_d = nc.dram_tensor("g0", [10, 10], FR, kind="ExternalInput")
    w1_d = nc.dram_tensor("w1", [8, 4, 128, C], BF, kind="ExternalInput")
    w2_d = nc.dram_tensor("w2", [4, 4, 128, C], BF, kind="ExternalInput")
    w3_d = nc.dram_tensor("w3", [4, 4, 128, C], BF, kind="ExternalInput")
    mw1_d = nc.dram_tensor("mw1", [4, 128, DMLP], FR, kind="ExternalInput")
    mw2_d = nc.dram_tensor("mw2", [16, 128, DMLP], BF, kind="ExternalInput")
    mw3_d = nc.dram_tensor("mw3", [128, DMLP], F32, kind="ExternalInput")
    mask_d = nc.dram_tensor("mask", [2, 2, 128, TN], FR, kind="ExternalInput")
    iota_d = nc.dram_tensor("iota", [128, TN + 1], F32, kind="ExternalInput")
    onesc_d = nc.dram_tensor("onesc", [128, 1], FR, kind="ExternalInput")
    onesr_d = nc.dram_tensor("onesr", [1, 128], FR, kind="ExternalInput")
    ones10_d = nc.dram_tensor("ones10", [10, 1], FR, kind="ExternalInput")
    flag_d = nc.dram_tensor("flag", [1, 1], F32, kind="ExternalInput")
    out_d = nc.dram_tensor("out", [128, C], F32, kind="ExternalOutput")

    with tile.TileContext(nc) as tc, nc.allow_low_precision(
        reason="bf16 weights / float32r matmul rounding are intentional"
    ):
        with (
            tc.tile_pool(name="pp", bufs=1) as pp,
            tc.tile_pool(name="hr", bufs=2) as hrp,
            tc.tile_pool(name="hq", bufs=2) as hqp,
            tc.tile_pool(name="srow", bufs=2) as srp,
            tc.tile_pool(name="dram", bufs=1, space="DRAM") as dp,
        ):
            # ---- persistent small tiles (sync queue) + early w2 (vector q)
            iota_sb = pp.tile([128, TN + 1], F32)
            nc.sync.dma_start(iota_sb[:], iota_d.ap())
            onesc = pp.tile([128, 1], FR)
            nc.sync.dma_start(onesc[:], onesc_d.ap())
            onesr = pp.tile([1, 128], FR)
            nc.sync.dma_start(onesr[:], onesr_d.ap())
            ones10 = pp.tile([10, 1], FR)
            nc.sync.dma_start(ones10[:], ones10_d.ap())
            w0sb = pp.tile([10, C], FR)
            nc.sync.dma_start(w0sb[:], w0_d.ap())
            g0sb = pp.tile([10, 10], FR)
            nc.sync.dma_start(g0sb[:], g0_d.ap())
            flag_sb = pp.tile([1, 1], F32)
            nc.sync.dma_start(flag_sb[:], flag_d.ap())
            onescb = pp.tile([128, 1], BF)
            nc.vector.tensor_copy(onescb[:], onesc[:])
            eps128 = pp.tile([128, 1], F32)
            nc.vector.memset(eps128[:], EPS)
            ident = pp.tile([128, 128], F32)
            make_identity(nc, ident[:])
            w2sb = pp.tile([128, 4, 4, C], BF)
            nc.gpsimd.dma_start(w2sb[:], w2_d.ap().rearrange("k c p f -> p k c f"))

            hp1 = pp.tile([128, 4, HP1_LEN], BF)
            hp2 = pp.tile([128, 4, HP2_LEN], BF)
            f_ct = pp.tile([128, 4, T3], FR)
            f_T = pp.tile([128, 2, C], FR)

            def norm_relu(psums, dst_fn, tw, sps, bps):
                """Channel-norm (mean==0 by weight centering) + relu for 4
                m-chunk psums of width tw."""
                ssq = sps.tile([1, 512], F32, tag="ssq")
                for m in range(4):
                    hq = hqp.tile([128, 512], BF, tag="hsq")
                    nc.scalar.activation(hq[:, :tw], psums[m], ACT.Square)
                    nc.tensor.matmul(
                        ssq[:, :tw], onescb[:], hq[:, :tw],
                        start=(m == 0), stop=(m == 3),
                    )
                sq = srp.tile([1, 512], F32, tag="sq")
                nc.scalar.activation(
                    sq[:, :tw], ssq[:, :tw], ACT.Sqrt,
                    bias=eps128[:1, :], scale=1.0 / (C - 1),
                )
                srow = srp.tile([1, 512], FR, tag="srow")
                nc.vector.reciprocal(srow[:, :tw], sq[:, :tw])
                sbc = bps.tile([128, 512], F32, tag="sbc")
                nc.tensor.matmul(
                    sbc[:, :tw], onesr[:], srow[:, :tw], start=True, stop=True
                )
                for m in range(4):
                    hr = hrp.tile([128, 512], F32, tag="hr")
                    nc.scalar.activation(hr[:, :tw], psums[m], ACT.Relu)
                    nc.vector.tensor_mul(dst_fn(m), hr[:, :tw], sbc[:, :tw])

            def conv_layer(wsb, src_views, dst, dst_off, taps, qmax, t_out,
                           cps, sps, bps):
                """Generic conv: wsb [128, K, 4, C] bf16; src_views[ci] strided
                [128, S, ext] fp32r; writes normed relu output to dst slices."""
                n_tot = taps * 4
                for t0, tw in _ttiles(t_out):
                    psums = []
                    for m in range(4):
                        ps = cps.tile([128, 512], F32, tag="cv")
                        n_mm = 0
                        for k in range(taps):
                            q, s = divmod(k, qmax)
                            for ci in range(4):
                                n_mm += 1
                                nc.tensor.matmul(
                                    ps[:, :tw],
                                    wsb[:, k, ci, m * 128 : (m + 1) * 128],
                                    src_views[ci][:, s, t0 + q : t0 + q + tw],
                                    start=(n_mm == 1),
                                    stop=(n_mm == n_tot),
                                )
                        psums.append(ps[:, :tw])
                    norm_relu(
                        psums,
                        lambda m, t0=t0, tw=tw: dst[
                            :, m, dst_off + t0 : dst_off + t0 + tw
                        ],
                        tw, sps, bps,
                    )
                    if t0 == 0 and dst_off > 0:
                        for e in range(dst_off):
                            nc.vector.tensor_copy(
                                dst[:, :, e : e + 1],
                                dst[:, :, 2 * dst_off - e : 2 * dst_off - e + 1],
                            )

            # =================== region A: conv0 + conv1 ===================
            with tc.tile_pool(name="ra", bufs=1) as ra:
                hp0 = ra.tile([128, 4, HP0_LEN], BF)
                w1sb = ra.tile([128, 8, 4, C], BF)
                nc.scalar.dma_start(
                    w1sb[:], w1_d.ap().rearrange("k c p f -> p k c f")
                )

                # ---------------- conv0 (Gram-trick norm)
                with (
                    tc.tile_pool(name="xc", bufs=2) as xcp,
                    tc.tile_pool(name="gsx", bufs=2) as gsxp,
                    tc.tile_pool(name="c0ps", bufs=2, space="PSUM") as cps0,
                    tc.tile_pool(name="gyps", bufs=2, space="PSUM") as gyp,
                    tc.tile_pool(name="gsps", bufs=2, space="PSUM") as gsp,
                    tc.tile_pool(name="gbps", bufs=2, space="PSUM") as gbp,
                ):
                    for t0, tw in _ttiles(T0):
                        xc = xcp.tile([10, 2560], FR, tag="xc")
                        nlo = 5 * tw - 4
                        nc.sync.dma_start(
                            xc[:, :nlo],
                            bass.AP(
                                tensor=xp_d, offset=5 * t0,
                                ap=[[1, 10], [1, nlo]],
                            ),
                        )
                        xv = xc[:].rearrange("p (t f) -> p f t", f=5)[:, 0, :tw]
                        # Gram path: ssq = ones10^T ((G @ X) * X)
                        y = gyp.tile([10, 512], F32, tag="gy")
                        nc.tensor.matmul(
                            y[:, :tw], g0sb[:], xv, start=True, stop=True
                        )
                        yx = gsxp.tile([10, 512], FR, tag="gsx")
                        nc.vector.tensor_mul(yx[:, :tw], y[:, :tw], xv)
                        ssq = gsp.tile([1, 512], F32, tag="gss")
                        nc.tensor.matmul(
                            ssq[:, :tw], ones10[:], yx[:, :tw],
                            start=True, stop=True,
                        )
                        sq = srp.tile([1, 512], F32, tag="sq")
                        nc.scalar.activation(
                            sq[:, :tw], ssq[:, :tw], ACT.Sqrt,
                            bias=eps128[:1, :], scale=1.0 / (C - 1),
                        )
                        srow = srp.tile([1, 512], FR, tag="srow")
                        nc.vector.reciprocal(srow[:, :tw], sq[:, :tw])
                        sbc = gbp.tile([128, 512], F32, tag="gbc")
                        nc.tensor.matmul(
                            sbc[:, :tw], onesr[:], srow[:, :tw],
                            start=True, stop=True,
                        )
                        for m in range(4):
                            ps = cps0.tile([128, 512], F32, tag="c0")
                            nc.tensor.matmul(
                                ps[:, :tw],
                                w0sb[:, m * 128 : (m + 1) * 128],
                                xv, start=True, stop=True,
                            )
                            hr = hrp.tile([128, 512], F32, tag="hr")
                            nc.scalar.activation(hr[:, :tw], ps[:, :tw], ACT.Relu)
                            nc.vector.tensor_mul(
                                hp0[:, m, 2 + t0 : 2 + t0 + tw],
                                hr[:, :tw], sbc[:, :tw],
                            )
                        if t0 == 0:
                            nc.vector.tensor_copy(hp0[:, :, 0:1], hp0[:, :, 4:5])
                            nc.vector.tensor_copy(hp0[:, :, 1:2], hp0[:, :, 3:4])

                if stage == 1:
                    dbg = ra.tile([128, C], F32)
                    nc.vector.tensor_copy(dbg[:], hp0[:, 0, :C])
                    nc.sync.dma_start(out_d.ap(), dbg[:])

                # ---------------- conv1
                if stage >= 2:
                    with (
                        tc.tile_pool(name="c1ps", bufs=4, space="PSUM") as cps,
                        tc.tile_pool(name="s1ps", bufs=2, space="PSUM") as sps,
                        tc.tile_pool(name="b1ps", bufs=2, space="PSUM") as bps,
                    ):
                        hp0v = [
                            hp0[:, ci, :].rearrange("p (t s) -> p s t", s=4)
                            for ci in range(4)
                        ]
                        conv_layer(w1sb, hp0v, hp1, 1, 8, 4, T1, cps, sps, bps)
            # region A closed: hp0, w1sb, xc freed

            if stage == 2:
                dbg = pp.tile([128, C], F32)
                nc.vector.tensor_copy(dbg[:], hp1[:, 0, :C])
                nc.sync.dma_start(out_d.ap(), dbg[:])

            # =================== region B ===================
            with tc.tile_pool(name="rb2", bufs=1) as rb2:
                mw2sb = rb2.tile([128, 16, DMLP], BF)
                nc.gpsimd.dma_start(
                    mw2sb[:], mw2_d.ap().rearrange("c p f -> p c f")
                )
                z1g = rb2.tile([128, 16, T3], BF)
                mask_sb = rb2.tile([128, 2, 2, TN], FR)
                nc.gpsimd.dma_start(
                    mask_sb[:], mask_d.ap().rearrange("a c p r -> p a c r")
                )
                mw3sb = rb2.tile([128, DMLP], F32)
                nc.gpsimd.dma_start(mw3sb[:], mw3_d.ap())

                # ------------ B1: conv2, conv3, transposes, mlp1
                with tc.tile_pool(name="rb1", bufs=1) as rb1:
                    w3sb = rb1.tile([128, 4, 4, C], BF)
                    nc.scalar.dma_start(
                        w3sb[:], w3_d.ap().rearrange("k c p f -> p k c f")
                    )
                    mw1sb = rb1.tile([128, 4, DMLP], FR)
                    nc.scalar.dma_start(
                        mw1sb[:], mw1_d.ap().rearrange("c p f -> p c f")
                    )

                    with (
                        tc.tile_pool(name="c2ps", bufs=4, space="PSUM") as cps,
                        tc.tile_pool(name="s2ps", bufs=2, space="PSUM") as sps,
                        tc.tile_pool(name="b2ps", bufs=2, space="PSUM") as bps,
                    ):
                        hp1v = [
                            hp1[:, ci, :].rearrange("p (t s) -> p s t", s=2)
                            for ci in range(4)
                        ]
                        conv_layer(w2sb, hp1v, hp2, 1, 4, 2, T2, cps, sps, bps)
                        if stage == 3:
                            dbg = rb1.tile([128, C], F32)
                            nc.vector.tensor_copy(dbg[:, :HP2_LEN], hp2[:, 0, :])
                            nc.sync.dma_start(out_d.ap(), dbg[:])
                        hp2v = [
                            hp2[:, ci, :].rearrange("p (t s) -> p s t", s=2)
                            for ci in range(4)
                        ]
                        if stage >= 4:
                            conv_layer(
                                w3sb, hp2v, f_ct, 0, 4, 2, T3, cps, sps, bps
                            )
                            # f_T[,:tch,:] = f_ct^T chunks via PE transpose
                            for ci in range(4):
                                for tch in range(2):
                                    tp = bps.tile([128, 512], F32, tag="sbc")
                                    nc.tensor.transpose(
                                        tp[:, :128],
                                        f_ct[
                                            :, ci, tch * 128 : (tch + 1) * 128
                                        ].bitcast(F32),
                                        ident[:],
                                    )
                                    nc.vector.tensor_copy(
                                        f_T[:, tch, ci * 128 : (ci + 1) * 128],
                                        tp[:, :128],
                                    )

                    if stage == 4:
                        nc.sync.dma_start(out_d.ap(), f_T[:, 0, :].bitcast(F32))

                    # ------------ mlp1 -> z1g [d1-chunk part, t free]
                    if stage >= 5:
                        with tc.tile_pool(name="zps", bufs=2, space="PSUM") as zps:
                            for j in range(16):
                                ps = zps.tile([128, T3], F32, tag="z")
                                for ci in range(4):
                                    nc.tensor.matmul(
                                        ps[:],
                                        mw1sb[:, ci, j * 128 : (j + 1) * 128],
                                        f_ct[:, ci, :],
                                        start=(ci == 0), stop=(ci == 3),
                                    )
                                nc.scalar.activation(z1g[:, j, :], ps[:], ACT.Gelu)
                # region B1 closed: w3sb, mw1sb freed

                # ------------ B3: z2 (z2^T), imp, warp, pooled, RS, norm4
                if stage >= 5:
                    with tc.tile_pool(name="rb3", bufs=1) as rb3:
                        z2g = rb3.tile([128, 2, DMLP], FR)
                        with tc.tile_pool(name="z2ps", bufs=2, space="PSUM") as z2ps:
                            for tch in range(2):
                                for dsp in range(4):
                                    ps = z2ps.tile([128, 512], F32, tag="z2")
                                    for d1c in range(16):
                                        nc.tensor.matmul(
                                            ps[:],
                                            z1g[:, d1c, tch * 128 : (tch + 1) * 128],
                                            mw2sb[:, d1c, dsp * 512 : (dsp + 1) * 512],
                                            start=(d1c == 0), stop=(d1c == 15),
                                        )
                                    nc.scalar.activation(
                                        z2g[:, tch, dsp * 512 : (dsp + 1) * 512],
                                        ps[:], ACT.Gelu,
                                    )

                        # logits: per-tch vector mul + free-reduce
                        logit = rb3.tile([128, 2], F32)
                        tmp3 = rb3.tile([128, DMLP], F32)
                        for tch in range(2):
                            nc.vector.tensor_mul(
                                tmp3[:], z2g[:, tch, :], mw3sb[:]
                            )
                            nc.vector.reduce_sum(
                                logit[:, tch : tch + 1], tmp3[:],
                                axis=mybir.AxisListType.X,
                            )
                        imp_loc = rb3.tile([128, 2], FR)
                        nc.scalar.activation(imp_loc[:], logit[:], ACT.Sigmoid)
                        nc.scalar.activation(
                            imp_loc[:], imp_loc[:], ACT.Identity,
                            bias=eps128[:, :],
                        )

                        with (
                            tc.tile_pool(name="tps", bufs=1, space="PSUM") as tps,
                            tc.tile_pool(name="cps3", bufs=2, space="PSUM") as cps3,
                            tc.tile_pool(name="pps", bufs=2, space="PSUM") as ppsp,
                        ):
                            # own sum -> AllReduce(pair) -> total
                            osp = tps.tile([1, 2], F32, tag="os")
                            nc.tensor.matmul(
                                osp[:], onesc[:], imp_loc[:], start=True, stop=True
                            )
                            os2 = rb3.tile([1, 2], F32)
                            nc.vector.tensor_copy(os2[:], osp[:])
                            own = rb3.tile([1, 1], F32)
                            nc.vector.tensor_tensor(
                                out=own[:], in0=os2[:, 0:1], in1=os2[:, 1:2],
                                op=mybir.AluOpType.add,
                            )
                            ar_in = dp.tile([1, 1], F32)
                            ar_out = dp.tile([1, 1], F32)
                            nc.sync.dma_start(ar_in[:], own[:])
                            nc.gpsimd.collective_compute(
                                "AllReduce",
                                mybir.AluOpType.add,
                                replica_groups=GROUPS,
                                ins=[ar_in[:]],
                                outs=[ar_out[:]],
                            )
                            tot = rb3.tile([1, 1], F32)
                            nc.sync.dma_start(tot[:], ar_out[:])

                            # offset = flag * (total - own); scale = TN / total
                            offsc = rb3.tile([1, 2], FR)
                            part = rb3.tile([1, 1], F32)
                            nc.vector.tensor_tensor(
                                out=part[:], in0=tot[:], in1=own[:],
                                op=mybir.AluOpType.subtract,
                            )
                            nc.vector.tensor_mul(part[:], part[:], flag_sb[:])
                            nc.vector.tensor_copy(offsc[:, 0:1], part[:])
                            rtot = rb3.tile([1, 1], F32)
                            nc.vector.reciprocal(rtot[:], tot[:])
                            nc.scalar.mul(rtot[:], rtot[:], float(TN))
                            nc.vector.tensor_copy(offsc[:, 1:2], rtot[:])
                            bcp = tps.tile([128, 2], F32, tag="bc")
                            nc.tensor.matmul(
                                bcp[:], onesr[:], offsc[:], start=True, stop=True
                            )
                            bc2 = rb3.tile([128, 2], F32)
                            nc.vector.tensor_copy(bc2[:], bcp[:])

                            if stage == 5:
                                nc.sync.dma_start(
                                    out_d.ap()[:, :2], imp_loc[:]
                                )
                                nc.sync.dma_start(
                                    out_d.ap()[:, 2:4], bc2[:]
                                )
                                nc.sync.dma_start(
                                    out_d.ap()[:, 4:512],
                                    z2g[:, 0, : 512 - 4].bitcast(F32),
                                )

                            # imp2: even-free moving operand [128, 2, 2]
                            zcol = rb3.tile([128, 2, 2], F32)
                            nc.vector.memset(zcol[:], 0.0)
                            imp2 = rb3.tile([128, 2, 2], FR)
                            nc.vector.tensor_copy(imp2[:], zcol[:])
                            for jc in range(2):
                                nc.vector.tensor_copy(
                                    imp2[:, jc, 0:1], imp_loc[:, jc : jc + 1]
                                )

                            # cs via masks; cs_f = (cs + off) * scale
                            wmat = []
                            dsA = []
                            for rc in range(2):
                                ds = []
                                for a in range(2):
                                    cp = cps3.tile([128, 2], F32, tag="cs")
                                    for jc in range(2):
                                        nc.tensor.matmul(
                                            cp[:],
                                            mask_sb[:, a, jc, rc * 128 : (rc + 1) * 128],
                                            imp2[:, jc, :],
                                            start=(jc == 0), stop=(jc == 1),
                                        )
                                    cst = rb3.tile([128, 1], F32, tag=f"cst{a}{rc}")
                                    nc.vector.tensor_scalar(
                                        out=cst[:],
                                        in0=cp[:, 0:1],
                                        scalar1=bc2[:, 0:1],
                                        scalar2=bc2[:, 1:2],
                                        op0=mybir.AluOpType.add,
                                        op1=mybir.AluOpType.mult,
                                    )
                                    tmp = rb3.tile([128, TN + 1], F32, tag="ptmp")
                                    nc.vector.tensor_scalar(
                                        out=tmp[:],
                                        in0=iota_sb[:],
                                        scalar1=cst[:],
                                        scalar2=None,
                                        op0=mybir.AluOpType.subtract,
                                    )
                                    pt = rb3.tile([128, TN + 1], F32, tag="prelu")
                                    nc.scalar.activation(
                                        pt[:], tmp[:], ACT.Relu, scale=-1.0
                                    )
                                    dt_ = rb3.tile([128, TN], F32, tag=f"d{a}")
                                    nc.vector.tensor_tensor(
                                        out=dt_[:],
                                        in0=pt[:, :TN],
                                        in1=pt[:, 1 : TN + 1],
                                        op=mybir.AluOpType.subtract,
                                    )
                                    ds.append(dt_)
                                wm = rb3.tile([128, TN], FR, tag=f"wm{rc}")
                                nc.vector.tensor_tensor(
                                    out=wm[:],
                                    in0=ds[0][:],
                                    in1=ds[1][:],
                                    op=mybir.AluOpType.subtract,
                                )
                                wmat.append(wm)

                            if stage == 7:
                                for rc in range(2):
                                    nc.sync.dma_start(
                                        out_d.ap()[:, rc * TN : (rc + 1) * TN],
                                        wmat[rc][:].bitcast(F32),
                                    )

                            # pooled partial = wmat^T @ f_T
                            pooled_sb = rb3.tile([128, 2, C], F32)
                            for nch in range(2):
                                pps = ppsp.tile([128, C], F32, tag="pool")
                                for rc in range(2):
                                    nc.tensor.matmul(
                                        pps[:],
                                        wmat[rc][:, nch * 128 : (nch + 1) * 128],
                                        f_T[:, rc, :],
                                        start=(rc == 0), stop=(rc == 1),
                                    )
                                nc.vector.tensor_copy(pooled_sb[:, nch, :], pps[:])
                            if stage == 8:
                                nc.sync.dma_start(out_d.ap(), pooled_sb[:, 0, :])

                            rs_in = dp.tile([2 * 128, C], F32)
                            nc.sync.dma_start(rs_in[:128, :], pooled_sb[:, 0, :])
                            nc.sync.dma_start(rs_in[128:, :], pooled_sb[:, 1, :])
                            rs_out = dp.tile([128, C], F32)
                            pr = rb3.tile([128, C], F32)
                            if stage >= 9:
                                nc.gpsimd.collective_compute(
                                    "ReduceScatter",
                                    mybir.AluOpType.add,
                                    replica_groups=GROUPS,
                                    ins=[rs_in[:]],
                                    outs=[rs_out[:]],
                                )
                                nc.sync.dma_start(pr[:], rs_out[:])
                            else:
                                nc.sync.dma_start(pr[:], rs_in[:128, :])

                            st6 = rb3.tile([128, 6], F32)
                            nc.vector.bn_stats(out=st6[:], in_=pr[:])
                            mv = rb3.tile([128, 2], F32)
                            nc.vector.bn_aggr(out=mv[:], in_=st6[:])
                            sd4 = rb3.tile([128, 1], F32)
                            nc.scalar.activation(
                                sd4[:], mv[:, 1:2], ACT.Sqrt,
                                bias=eps128[:], scale=float(C) / (C - 1),
                            )
                            rstd = rb3.tile([128, 1], F32)
                            nc.vector.reciprocal(rstd[:], sd4[:])
                            zt = rb3.tile([128, C], F32)
                            nc.vector.tensor_scalar(
                                out=zt[:],
                                in0=pr[:],
                                scalar1=mv[:, 0:1],
                                scalar2=rstd[:],
                                op0=mybir.AluOpType.subtract,
                                op1=mybir.AluOpType.mult,
                            )
                            out_sb = rb3.tile([128, C], F32)
                            nc.scalar.activation(out_sb[:], zt[:], ACT.Relu)
                            if stage >= 9:
                                nc.sync.dma_start(out_d.ap(), out_sb[:])

    nc.compile()
    _CACHE[key] = nc
    return nc


# ---------------------------------------------------------------- entrypoint
def _prepare_in_maps(inputs):
    import ml_dtypes

    bf16 = ml_dtypes.bfloat16
    x = np.asarray(inputs["x"], np.float32)
    conv_ws = [np.asarray(inputs[f"conv{i}_w"], np.float32) for i in range(4)]
    ws_h = _prep_conv_weights(conv_ws, bf16)
    mw1 = np.ascontiguousarray(
        np.asarray(inputs["mlp_w1"], np.float32).reshape(4, 128, DMLP)
    )
    mw2 = np.ascontiguousarray(
        np.asarray(inputs["mlp_w2"], np.float32).reshape(16, 128, DMLP).astype(bf16)
    )
    mw3 = np.ascontiguousarray(
        np.broadcast_to(
            np.asarray(inputs["mlp_w3"], np.float32).reshape(1, DMLP), (128, DMLP)
        )
    )
    xs = _prep_x_slices(x)
    iota = _prep_iota()
    masks = [_prep_masks(h) for h in range(2)]
    onesc = np.ones((128, 1), np.float32)
    onesr = np.ones((1, 128), np.float32)
    ones10 = np.ones((10, 1), np.float32)

    in_maps = []
    for core in range(8):
        b, h = core // 2, core % 2
        (w0, w1, w2, w3), g0 = ws_h[h]
        in_maps.append(
            {
                "xp": xs[b][h],
                "w0": w0,
                "g0": g0,
                "w1": w1,
                "w2": w2,
                "w3": w3,
                "mw1": mw1,
                "mw2": mw2,
                "mw3": mw3,
                "mask": masks[h],
                "iota": iota,
                "onesc": onesc,
                "onesr": onesr,
                "ones10": ones10,
                "flag": np.full((1, 1), float(h), np.float32),
            }
        )
    return in_maps


def _postprocess(results):
    out = np.empty((B, C, TN), np.float32)
    for b in range(B):
        rows = np.concatenate([results[2 * b]["out"], results[2 * b + 1]["out"]], 0)
        out[b] = rows.T
    return out


def kernel(**inputs) -> np.ndarray:
    if not _fast_path_ok(inputs):
        return _np_reference(inputs)
    in_maps = _prepare_in_maps(inputs)
    nc = _build_program()
    res = run_bass_kernel_spmd(nc, in_maps, core_ids=list(range(8)))
    return _postprocess(res.results)
